# revision 1
# baseline (speedup 1.0000x reference)
"""Entmax-1.5 via monotone sqrt-Newton iterations on Trainium2.

Data-parallel over 8 NeuronCores: X [8, 2048, 4096] is sharded on the
leading dims (2048 rows x 4096 per core); the reduction dim stays local;
no communication. Input is downconverted to fp16 on the host (bisection
tolerances dwarf fp16 noise); output computed in fp16, upcast on host.

Math (per row; r = relu(x - th); p_out = r^2/sum(r^2) — the alpha-1
scaling cancels in the normalization):
  th0 = rowmax(x) - 2              # provably below the root
  r = relu(x - th0)
  repeat ITERS-1 times:            # Newton on u(th) = ||r||_2:
    S2 = sum r^2; h = sum r        # u is convex decreasing =>
    dm = (sqrt(S2)-2)*sqrt(S2)/h   # steps from below never overshoot =>
    r  = relu(r - dm)              # exact fp16 r-recycling, x read once
  out = r^2 / S2

Engine assignment per iteration (accumulating ops run at 1x ~4.4us/tile
on DVE and ~4.0us on ACT — they dominate; non-accum fp16 tensor_scalar
runs 4x ~1.3us): every tile needs a state pass + an S2 accum + an h
accum. Tiles rotate between: A (DVE state + DVE STT square-accum),
C (DVE state + ACT Square-accum), D (ACT Prelu = state+h in one op +
ACT Square-accum); C/A tiles compute h via a DVE tensor_tensor add-tree
(3 levels at 2x/4x + short 1x accum = ~2.9us vs 4.4 flat). Per-row
scalars are batched in flat [P, nt] tiles; the Newton update is ~5
small ops per 4-tile group.
"""

import os

import numpy as np

import concourse.bass as bass  # noqa: F401
import concourse.tile as tile
from concourse import bacc, mybir
from concourse.bass_utils import run_bass_kernel_spmd

N_CORES = 8
D = 4096
P = 128

ITERS = int(os.environ.get("K_ITERS", "4"))     # eval passes (incl. th0)
NG = int(os.environ.get("K_NG", "4"))           # scalar-batch groups
SQ_MODE = os.environ.get("K_SQ", "stt")         # stt | custom (DVE square)
# per-iteration tile-type counts (of nt=16), rotated over tiles:
#  A: DVE state + DVE STT-S2-accum + DVE h-tree
#  C: DVE state + DVE h-tree + ACT Square-accum (S2)
#  D: ACT Prelu-accum (state + h) + ACT Square-accum (S2)
A_CNT = [int(c, 16) for c in os.environ.get("K_ACNT", "1114")]
D_CNT = [int(c, 16) for c in os.environ.get("K_DCNT", "d000")]
H_TREE = os.environ.get("K_HTREE", "1") == "1"
# per-iteration count of C-tiles whose state pass runs on GPSIMD
G_CNT = [int(c, 16) for c in os.environ.get("K_GCNT", "0")]
# final-norm mult tiles per group on DVE (rest ACT Copy-scale)
FIN_DVE = int(os.environ.get("K_FIN", "10"))

TRACE = False
LAST_RESULT = None

_NC_CACHE = {}


def _register_dve_op(op_name, spec):
    from concourse import dve_ops as DO
    from concourse.dve_spec import lower, _has_src1 as has_src1
    from concourse.dve_uop import DveOpSpec

    for o in DO.OPS:
        if o.name == op_name:
            return o
    row = DO._CUSTOM_DVE_ROW_BASE + len(DO.OPS)
    assert row < 0x20
    shas = {}
    for ver in ("v3", "v4"):
        s = DveOpSpec(name=op_name, opcode=row, uops=lower(spec, ver=ver),
                      rd1_en=has_src1(spec))
        shas[ver] = s.sha(ver)
    op = DO.DveOp(op_name, spec, subdim=False, uops_sha=shas)
    DO.OPS.append(op)
    DO._SUB_OPCODE_FOR_NAME[op_name] = row
    DO.CUSTOM_DVE_SPECS[op_name] = spec
    return op


def _get_sq_op():
    from operator import add as _op_add

    from concourse.dve_spec import Spec, Src0, sq

    def _ref(in0, in1, c0, c1, c2):
        b = (in0.astype(np.float32) ** 2).astype(np.float32)
        return b, b.reshape(b.shape[0], -1).sum(axis=-1, keepdims=True)

    return _register_dve_op(
        "ENTMAX_SQ_ANT",
        Spec(body=sq(Src0), accum=_op_add, reference=_ref),
    )


def _build(rows: int):
    f32 = mybir.dt.float32
    f16 = mybir.dt.float16
    OP = mybir.AluOpType
    AF = mybir.ActivationFunctionType
    AX = mybir.AxisListType

    nc = bacc.Bacc(None, target_bir_lowering=False)
    Xd = nc.declare_dram_parameter("X", [rows, D], f16, isOutput=False)
    Od = nc.declare_dram_parameter("OUT", [rows, D], f16, isOutput=True)
    nt = rows // P
    gt = nt // NG  # tiles per group

    with tile.TileContext(nc) as tc:
        with (
            tc.tile_pool(name="xp", bufs=3) as xp,       # f16 staging
            tc.tile_pool(name="rp", bufs=nt + 1) as rp,  # fp16 r/q/p
            tc.tile_pool(name="sv", bufs=1) as sv,       # DVE sq scratch
            tc.tile_pool(name="tp", bufs=2) as tp,       # h-tree scratch
            tc.tile_pool(name="sa", bufs=1) as sa,       # ACT sq scratch
            tc.tile_pool(name="st", bufs=9) as st,       # [P, nt] scalars
        ):
            MX = st.tile([P, nt], f32, tag="mx", name="mx")
            TH = st.tile([P, nt], f32, tag="th", name="th")
            H = st.tile([P, nt], f32, tag="h", name="h")
            S2 = st.tile([P, nt], f32, tag="s2", name="s2")
            U = st.tile([P, nt], f32, tag="u", name="u")
            IH = st.tile([P, nt], f32, tag="ih", name="ih")
            TM = st.tile([P, nt], f32, tag="tm", name="tm")
            DM = st.tile([P, nt], f32, tag="dm", name="dm")
            RS = st.tile([P, nt], f32, tag="rs", name="rs")
            NTH = st.tile([P, nt], f32, tag="nth", name="nth")
            NDM = st.tile([P, nt], f32, tag="ndm", name="ndm")

            xt, rt = {}, {}

            def gs(t, g):  # group slice
                return t[:, g * gt:(g + 1) * gt]

            def emit_dma(j):
                xt[j] = xp.tile([P, D], f16, tag="xt", name=f"x{j}")
                nc.sync.dma_start(out=xt[j][:],
                                  in_=Xd[j * P:(j + 1) * P, :])

            def emit_max(j):
                nc.vector.reduce_max(MX[:, j:j + 1], xt[j][:], axis=AX.X)

            def emit_th0(j):
                # small guard: keep th0 strictly below the root
                nc.vector.tensor_scalar(TH[:, j:j + 1], MX[:, j:j + 1], 2.002,
                                        None, OP.subtract)
                nc.vector.tensor_scalar(NTH[:, j:j + 1], MX[:, j:j + 1], -1.0,
                                        2.002, OP.mult, OP.add)

            def tile_type(j, k):
                jj = (j + 5 * k) % nt
                na = A_CNT[k] if k < len(A_CNT) else A_CNT[-1]
                nd = D_CNT[k] if k < len(D_CNT) else D_CNT[-1]
                if jj < na:
                    return "A"
                if jj < na + nd:
                    return "D"
                return "C"

            def emit_h_tree(j, rnew):
                hc = H[:, j:j + 1]
                if not H_TREE:
                    hs = sv.tile([P, D], f16, tag="q", name=f"hs{j}")
                    nc.vector.tensor_scalar(hs[:], rnew[:], 0.0, None,
                                            OP.max, OP.add, accum_out=hc)
                    return
                t1 = tp.tile([P, D // 2], f16, tag="t1", name=f"t1_{j}")
                nc.vector.tensor_add(t1[:], rnew[:, :D // 2],
                                     rnew[:, D // 2:])
                t2 = tp.tile([P, D // 4], f16, tag="t1", name=f"t2_{j}")
                nc.vector.tensor_add(t2[:], t1[:, :D // 4], t1[:, D // 4:])
                t3 = tp.tile([P, D // 8], f16, tag="t1", name=f"t3_{j}")
                nc.vector.tensor_add(t3[:], t2[:, :D // 8], t2[:, D // 8:])
                t4 = tp.tile([P, D // 8], f16, tag="t1", name=f"t4_{j}")
                nc.vector.tensor_scalar(t4[:], t3[:], 0.0, None, OP.max,
                                        OP.add, accum_out=hc)

            def emit_iter(j, k):
                last = k == ITERS - 1
                hc = H[:, j:j + 1]
                sc = S2[:, j:j + 1]
                tt = tile_type(j, k)
                src = xt[j] if k == 0 else rt[j]
                scal = TH[:, j:j + 1] if k == 0 else DM[:, j:j + 1]
                rnew = rp.tile([P, D], f16, tag="rt", name=f"r{j}_{k}")
                if tt == "D":
                    # ACT Prelu: state + h in one pass (alpha=0 => relu);
                    # bias is the negated threshold column
                    nbias = (NTH if k == 0 else NDM)[:, j:j + 1]
                    nc.scalar.activation(
                        rnew[:], src[:], AF.Prelu, bias=nbias,
                        scale=1.0, alpha=0.0,
                        accum_out=None if last else hc)
                else:
                    ng_ = G_CNT[k] if k < len(G_CNT) else G_CNT[-1]
                    jj = (j + 5 * k + 2) % nt
                    eng = nc.gpsimd if jj < ng_ else nc.vector
                    eng.tensor_scalar(rnew[:], src[:], scal, scal,
                                      OP.max, OP.subtract)
                rt[j] = rnew
                # square pass: q = r^2, S2 = sum q
                if last:
                    q = rp.tile([P, D], f16, tag="rt", name=f"q{j}")
                else:
                    q = (sv if tt == "A" else sa).tile(
                        [P, D], f16, tag="q", name=f"qs{j}_{k}")
                if tt == "A":
                    nc.vector.scalar_tensor_tensor(
                        q[:], rnew[:], 1.0, rnew[:], OP.mult, OP.mult,
                        accum_out=sc)
                else:
                    nc.scalar.activation(q[:], rnew[:], AF.Square, bias=0.0,
                                         scale=1.0, accum_out=sc)
                if last:
                    rt[j] = q  # keep q = r^2 for the final normalize
                    return
                if tt != "D":
                    emit_h_tree(j, rnew)

            def emit_update(g):
                # dm = (u - 2) * u / h  from S2, h
                nc.scalar.activation(gs(U, g), gs(S2, g), AF.Sqrt)
                nc.vector.reciprocal(gs(IH, g), gs(H, g))
                nc.vector.tensor_mul(gs(TM, g), gs(U, g), gs(IH, g))
                nc.vector.scalar_tensor_tensor(
                    gs(DM, g), gs(U, g), -2.0, gs(TM, g), OP.add, OP.mult)
                nc.vector.tensor_scalar(gs(NDM, g), gs(DM, g), -1.0, None,
                                        OP.mult)

            def emit_recip_s2(g):
                nc.vector.reciprocal(gs(RS, g), gs(S2, g))

            def emit_final(j):
                p = rp.tile([P, D], f16, tag="rt", name=f"p{j}")
                rsc = RS[:, j:j + 1]
                if (j * 5) % nt < FIN_DVE:
                    nc.vector.tensor_scalar(p[:], rt[j][:], rsc, None,
                                            OP.mult)
                else:
                    nc.scalar.activation(p[:], rt[j][:], AF.Copy, bias=0.0,
                                         scale=rsc)
                nc.gpsimd.dma_start(out=Od[j * P:(j + 1) * P, :], in_=p[:])

            # ---- schedule (wave order; Tile inserts cross-engine deps) ---
            for g in range(NG):
                for jj in range(gt):
                    j = g * gt + jj
                    emit_dma(j)
                    emit_max(j)
                    emit_th0(j)
                    emit_iter(j, 0)
                if ITERS > 1:
                    emit_update(g)
            for k in range(1, ITERS):
                for g in range(NG):
                    for jj in range(gt):
                        emit_iter(g * gt + jj, k)
                    if k < ITERS - 1:
                        emit_update(g)
                    else:
                        emit_recip_s2(g)
                        for jj in range(gt):
                            emit_final(g * gt + jj)

    nc.finalize()
    return nc


def _get_nc(rows: int):
    key = (rows, ITERS, NG, tuple(A_CNT), tuple(D_CNT), FIN_DVE, SQ_MODE,
           H_TREE, tuple(G_CNT))
    if key not in _NC_CACHE:
        _NC_CACHE[key] = _build(rows)
    return _NC_CACHE[key]


def _ensure_ntff_hook():
    """Register the NTFF profile hook that bass_utils needs for trace=True
    under axon (this image's antenv lacks axon_hooks; build it from the
    boot shim's ctypes driver). Also neuter the S3 artifact upload."""
    import sys as _sys
    import types

    import antenv
    import concourse.bass_utils as _bu

    _bu.upload_artifacts = lambda tmpdir: str(tmpdir)
    try:
        from antenv import axon_hooks  # noqa: F401
        return
    except ImportError:
        pass
    from trn_agent_boot.trn_boot import _ntff_profile_via_ctypes

    hook = _ntff_profile_via_ctypes("/opt/axon/libaxon_pjrt.so")
    mod = types.ModuleType("antenv.axon_hooks")
    mod._hook = hook
    mod.get_axon_ntff_profile_hook = lambda: mod._hook
    mod.set_axon_ntff_profile_hook = lambda h: setattr(mod, "_hook", h)
    _sys.modules["antenv.axon_hooks"] = mod
    antenv.axon_hooks = mod


def kernel(X, alpha):
    global LAST_RESULT
    X = np.asarray(X, dtype=np.float32)
    a = float(np.asarray(alpha, dtype=np.float32).reshape(()))
    # this implementation hardcodes the alpha=1.5 exponent (squares)
    assert abs(a - 1.5) < 1e-6, f"unsupported alpha={a}"

    orig_shape = X.shape
    Xf = np.ascontiguousarray(X.reshape(-1, D))
    rows_total = Xf.shape[0]
    assert rows_total % N_CORES == 0
    rows = rows_total // N_CORES
    shards = np.split(Xf, N_CORES, axis=0)

    nc = _get_nc(rows)
    in_maps = [{"X": np.ascontiguousarray(s.astype(np.float16))}
               for s in shards]
    if TRACE:
        _ensure_ntff_hook()
    res = None
    for attempt in range(3):
        try:
            res = run_bass_kernel_spmd(nc, in_maps, list(range(N_CORES)),
                                       trace=TRACE)
            break
        except Exception:
            if attempt == 2:
                raise
            import time
            time.sleep(5.0)
    LAST_RESULT = res
    out = np.concatenate([np.asarray(r["OUT"]) for r in res.results], axis=0)
    return np.ascontiguousarray(
        out.astype(np.float32).reshape(orig_shape))



# revision 2
# speedup vs baseline: 1.2703x; 1.2703x over previous
"""Entmax-1.5 via folded-max solve + one full-data polish on Trainium2.

Data-parallel over 8 NeuronCores: X [8, 2048, 4096] sharded on leading
dims (2048 rows x 4096 per core), reduction dim local, no comms. fp16
on-device (host converts); output upcast on host.

Math per row (x = raw X row; th = 2*tau; the alpha scaling cancels):
  solve  u(th)^2 = sum relu(x - th)^2 = 4,  p = relu(x-th)^2 / S2.
Per [128, 4096] tile:
  1. max-fold tree (tensor_tensor max): z = bucket maxima [128, 256];
     rowmax M from z; th0 = M - 2.002 (always below root).
  2. K Newton-on-u iters on the folded problem (custom DVE RSQ op gives
     A + r^2; ACT Prelu-accum gives B). Folds only lose mass => the
     folded root underestimates => monotone from below, B > 1.9 always.
  3. One full-data polish: A_full via 1 pass (DVE RSQ or ACT
     Prelu+Square); B_est = B_fold*sqrt(A_full/A_fold). Newton -> th_f.
  4. Final: q = relu(x-th_f)^2 with accum S2, p = q/S2, DMA out.

Schedule: 4 superblocks of 4 tiles rotate through phases [trees,
folded k=0..K-1, polish, finals]; each round emits eval work first,
dense bulk second, scalar update chains last, so the in-order engines
always have bulk work queued while cross-engine chains settle.
"""

import os

import numpy as np

import concourse.bass as bass  # noqa: F401
import concourse.tile as tile
from concourse import bacc, mybir
from concourse.bass_utils import run_bass_kernel_spmd

N_CORES = 8
D = 4096
P = 128
FB = 256          # folded width per tile (fold factor 16)

K_ITERS = int(os.environ.get("K_ITERS", "3"))    # folded Newton iters
GT = int(os.environ.get("K_GT", "4"))            # tiles per superblock
K_POL = os.environ.get("K_POL", "sqrt1")         # sqrt1 | full
PACT = int(os.environ.get("K_PACT", "8"))        # polish-A tiles on ACT /16
FACT = int(os.environ.get("K_FACT", "9"))        # final tiles on ACT /16
SACT = int(os.environ.get("K_SACT", "0"))        # scale tiles on ACT /16
GUARD = float(os.environ.get("K_GUARD", "2.002"))

TRACE = False
LAST_RESULT = None

_NC_CACHE = {}


def _register_dve_op(op_name, spec):
    from concourse import dve_ops as DO
    from concourse.dve_spec import lower, _has_src1 as has_src1
    from concourse.dve_uop import DveOpSpec

    for o in DO.OPS:
        if o.name == op_name:
            return o
    row = DO._CUSTOM_DVE_ROW_BASE + len(DO.OPS)
    assert row < 0x20
    shas = {}
    for ver in ("v3", "v4"):
        s = DveOpSpec(name=op_name, opcode=row, uops=lower(spec, ver=ver),
                      rd1_en=has_src1(spec))
        shas[ver] = s.sha(ver)
    op = DO.DveOp(op_name, spec, subdim=False, uops_sha=shas)
    DO.OPS.append(op)
    DO._SUB_OPCODE_FOR_NAME[op_name] = row
    DO.CUSTOM_DVE_SPECS[op_name] = spec
    return op


def _get_rsq_op():
    """out = relu(in0 - s0)^2 ; accum_out = sum(out)."""
    from operator import add as _op_add

    from concourse.dve_spec import Spec, Src0, C0, relu, sq

    def _ref(in0, in1, c0, c1, c2):
        r = np.maximum(in0.astype(np.float32) - c0, 0.0)
        b = (r * r).astype(np.float32)
        return b, b.reshape(b.shape[0], -1).sum(axis=-1, keepdims=True)

    return _register_dve_op(
        "ENTMAX_RSQ_ANT",
        Spec(body=sq(relu(Src0 - C0)), accum=_op_add, reference=_ref),
    )


def _build(rows: int):
    f32 = mybir.dt.float32
    f16 = mybir.dt.float16
    OP = mybir.AluOpType
    AF = mybir.ActivationFunctionType
    AX = mybir.AxisListType

    RSQ = _get_rsq_op()

    nc = bacc.Bacc(None, target_bir_lowering=False)
    Xd = nc.declare_dram_parameter("X", [rows, D], f16, isOutput=False)
    Od = nc.declare_dram_parameter("OUT", [rows, D], f16, isOutput=True)
    nt = rows // P                      # 16 tiles
    ng = nt // GT                       # superblocks
    K = K_ITERS

    def spread(j, cnt, off=0):
        return ((j * 5 + off) % nt) < cnt

    with tile.TileContext(nc) as tc:
        with (
            tc.tile_pool(name="xp", bufs=nt) as xp,
            tc.tile_pool(name="t1p", bufs=1) as t1p,
            tc.tile_pool(name="t2p", bufs=2) as t2p,
            tc.tile_pool(name="t3p", bufs=2) as t3p,
            tc.tile_pool(name="zp", bufs=1) as zp,      # folded Z
            tc.tile_pool(name="za", bufs=2) as zap,     # DVE folded scratch
            tc.tile_pool(name="zb", bufs=2) as zbp,     # ACT folded scratch
            tc.tile_pool(name="pf", bufs=1) as pfp,     # DVE polish scratch
            tc.tile_pool(name="pa", bufs=2) as pap,     # ACT polish scratch
            tc.tile_pool(name="qp", bufs=2) as qp,      # final q
            tc.tile_pool(name="pp", bufs=2) as ppool,   # final p
            tc.tile_pool(name="st", bufs=1) as st,      # [P, nt] scalars
        ):
            Z = zp.tile([P, nt * FB], f16, tag="z", name="Z")
            MX = st.tile([P, nt], f32, tag="mx", name="mx")
            TH = [st.tile([P, nt], f32, tag=f"th{k}", name=f"th{k}")
                  for k in range(K + 2)]
            NTH = [st.tile([P, nt], f32, tag=f"nth{k}", name=f"nth{k}")
                   for k in range(K + 2)]
            AA = [st.tile([P, nt], f32, tag=f"a{k}", name=f"a{k}")
                  for k in range(K)]
            BB = [st.tile([P, nt], f32, tag=f"b{k}", name=f"b{k}")
                  for k in range(K)]
            AZ = st.tile([P, nt], f32, tag="az", name="az")
            BZ = st.tile([P, nt], f32, tag="bz", name="bz")
            AP_ = st.tile([P, nt], f32, tag="ap", name="ap")
            BPE = st.tile([P, nt], f32, tag="bpe", name="bpe")
            S2 = st.tile([P, nt], f32, tag="s2", name="s2")
            RS = st.tile([P, nt], f32, tag="rs", name="rs")
            tmp = {}

            def stt(name):
                if name not in tmp:
                    tmp[name] = st.tile([P, nt], f32, tag=name, name=name)
                return tmp[name]

            xt = {}

            def gs(t, g):
                return t[:, g * GT:(g + 1) * GT]

            def cs(t, j):
                return t[:, j:j + 1]

            def zs(j):
                return Z[:, j * FB:(j + 1) * FB]

            def emit_dma(j):
                xt[j] = xp.tile([P, D], f16, tag="xt", name=f"x{j}")
                nc.sync.dma_start(out=xt[j][:], in_=Xd[j * P:(j + 1) * P, :])

            def emit_tree(j):
                t1 = t1p.tile([P, D // 2], f16, tag="t1", name=f"t1_{j}")
                nc.vector.tensor_tensor(t1[:], xt[j][:, :D // 2],
                                        xt[j][:, D // 2:], OP.max)
                t2 = t2p.tile([P, D // 4], f16, tag="t2", name=f"t2_{j}")
                nc.vector.tensor_tensor(t2[:], t1[:, :D // 4], t1[:, D // 4:],
                                        OP.max)
                t3 = t3p.tile([P, D // 8], f16, tag="t3", name=f"t3_{j}")
                nc.vector.tensor_tensor(t3[:], t2[:, :D // 8], t2[:, D // 8:],
                                        OP.max)
                nc.vector.tensor_tensor(zs(j), t3[:, :FB], t3[:, FB:], OP.max)
                nc.vector.reduce_max(cs(MX, j), zs(j), axis=AX.X)

            def emit_th0(g):
                nc.vector.tensor_scalar(gs(TH[0], g), gs(MX, g), GUARD, None,
                                        OP.subtract)
                nc.vector.tensor_scalar(gs(NTH[0], g), gs(MX, g), -1.0, GUARD,
                                        OP.mult, OP.add)

            def emit_folded_evals(g, k, acc_a, acc_b, th_k, nth_k, sfx):
                for jj in range(GT):
                    j = g * GT + jj
                    sa = zap.tile([P, FB], f16, tag="za", name=f"za{j}_{sfx}")
                    nc.vector._custom_dve(RSQ, out=sa[:], in0=zs(j),
                                          s0=cs(th_k, j),
                                          accum_out=cs(acc_a, j))
                    sb = zbp.tile([P, FB], f16, tag="zb", name=f"zb{j}_{sfx}")
                    nc.scalar.activation(sb[:], zs(j), AF.Prelu,
                                         bias=cs(nth_k, j), scale=1.0,
                                         alpha=0.0, accum_out=cs(acc_b, j))

            def emit_folded_update(g, k):
                # newton on u: th += (u - 2) * u / B ; also negate for ACT
                U = stt(f"u{k}")
                nc.scalar.activation(gs(U, g), gs(AA[k], g), AF.Sqrt)
                IB = stt(f"ib{k}")
                nc.vector.reciprocal(gs(IB, g), gs(BB[k], g))
                TM = stt(f"tm{k}")
                nc.vector.tensor_mul(gs(TM, g), gs(U, g), gs(IB, g))
                DM = stt(f"dm{k}")
                nc.vector.scalar_tensor_tensor(gs(DM, g), gs(U, g), -2.0,
                                               gs(TM, g), OP.add, OP.mult)
                nc.vector.tensor_add(gs(TH[k + 1], g), gs(TH[k], g),
                                     gs(DM, g))
                nc.vector.tensor_scalar(gs(NTH[k + 1], g), gs(TH[k + 1], g),
                                        -1.0, None, OP.mult)

            def emit_polish_a(j):
                if spread(j, PACT, 2):
                    r = pap.tile([P, D], f16, tag="pa", name=f"pr{j}")
                    nc.scalar.activation(r[:], xt[j][:], AF.Prelu,
                                         bias=cs(NTH[K], j), scale=1.0,
                                         alpha=0.0,
                                         accum_out=cs(BPE, j)
                                         if K_POL == "full" else None)
                    q = pap.tile([P, D], f16, tag="pa", name=f"pq{j}")
                    nc.scalar.activation(q[:], r[:], AF.Square, bias=0.0,
                                         scale=1.0, accum_out=cs(AP_, j))
                else:
                    s = pfp.tile([P, D], f16, tag="pf", name=f"pf{j}")
                    nc.vector._custom_dve(RSQ, out=s[:], in0=xt[j][:],
                                          s0=cs(TH[K], j),
                                          accum_out=cs(AP_, j))

            def emit_polish_update(g):
                U = stt("pu")
                nc.scalar.activation(gs(U, g), gs(AP_, g), AF.Sqrt)
                if K_POL == "sqrt1":
                    IAZ = stt("iaz")
                    nc.vector.reciprocal(gs(IAZ, g), gs(AZ, g))
                    RT = stt("rt")
                    nc.vector.tensor_mul(gs(RT, g), gs(AP_, g), gs(IAZ, g))
                    SR = stt("sr")
                    nc.scalar.activation(gs(SR, g), gs(RT, g), AF.Sqrt)
                    BE = stt("be")
                    nc.vector.tensor_mul(gs(BE, g), gs(BZ, g), gs(SR, g))
                else:
                    BE = BPE
                IB = stt("pib")
                nc.vector.reciprocal(gs(IB, g), gs(BE, g))
                TM = stt("ptm")
                nc.vector.tensor_mul(gs(TM, g), gs(U, g), gs(IB, g))
                DM = stt("pdm")
                nc.vector.scalar_tensor_tensor(gs(DM, g), gs(U, g), -2.0,
                                               gs(TM, g), OP.add, OP.mult)
                nc.vector.tensor_add(gs(TH[K + 1], g), gs(TH[K], g),
                                     gs(DM, g))
                nc.vector.tensor_scalar(gs(NTH[K + 1], g), gs(TH[K + 1], g),
                                        -1.0, None, OP.mult)

            def emit_final_scale_out(j):
                q = qp.tile([P, D], f16, tag="q", name=f"q{j}")
                if spread(j, FACT, 3):
                    r = pap.tile([P, D], f16, tag="pa", name=f"fr{j}")
                    nc.scalar.activation(r[:], xt[j][:], AF.Prelu,
                                         bias=cs(NTH[K + 1], j), scale=1.0,
                                         alpha=0.0)
                    nc.scalar.activation(q[:], r[:], AF.Square, bias=0.0,
                                         scale=1.0, accum_out=cs(S2, j))
                else:
                    nc.vector._custom_dve(RSQ, out=q[:], in0=xt[j][:],
                                          s0=cs(TH[K + 1], j),
                                          accum_out=cs(S2, j))
                nc.vector.reciprocal(cs(RS, j), cs(S2, j))
                p = ppool.tile([P, D], f16, tag="p", name=f"p{j}")
                if spread(j, SACT, 4):
                    nc.scalar.activation(p[:], q[:], AF.Copy, bias=0.0,
                                         scale=cs(RS, j))
                else:
                    nc.vector.tensor_scalar(p[:], q[:], cs(RS, j), None,
                                            OP.mult)
                nc.gpsimd.dma_start(out=Od[j * P:(j + 1) * P, :], in_=p[:])

            # ---------------- rotating-phase schedule ----------------
            # phases per superblock g: 0 trees, 1..K folded iters,
            # K+1 extra evals + polish-A, K+2 polish update + finals.
            for j in range(nt):
                emit_dma(j)
            NPH = K + 3
            for r in range(NPH + ng - 1):
                # part 1: eval work (feeds cross-engine chains)
                for g in range(ng):
                    p = r - g
                    if 1 <= p <= K:
                        k = p - 1
                        emit_folded_evals(g, k, AA[k], BB[k], TH[k], NTH[k],
                                          f"i{k}")
                    elif p == K + 1:
                        emit_folded_evals(g, K, AZ, BZ, TH[K], NTH[K], "x")
                # part 2: dense bulk
                for g in range(ng):
                    p = r - g
                    if p == 0:
                        for jj in range(GT):
                            emit_tree(g * GT + jj)
                        emit_th0(g)
                    elif p == K + 1:
                        for jj in range(GT):
                            emit_polish_a(g * GT + jj)
                    elif p == K + 2:
                        emit_polish_update(g)
                        for jj in range(GT):
                            emit_final_scale_out(g * GT + jj)
                # part 3: scalar update chains
                for g in range(ng):
                    p = r - g
                    if 1 <= p <= K:
                        emit_folded_update(g, p - 1)

    nc.finalize()
    return nc


def _get_nc(rows: int):
    key = (rows, K_ITERS, GT, K_POL, PACT, FACT, SACT)
    if key not in _NC_CACHE:
        _NC_CACHE[key] = _build(rows)
    return _NC_CACHE[key]


def _ensure_ntff_hook():
    """Register the NTFF profile hook for trace=True under axon."""
    import sys as _sys
    import types

    import antenv
    import concourse.bass_utils as _bu

    _bu.upload_artifacts = lambda tmpdir: str(tmpdir)
    try:
        from antenv import axon_hooks  # noqa: F401
        return
    except ImportError:
        pass
    from trn_agent_boot.trn_boot import _ntff_profile_via_ctypes

    hook = _ntff_profile_via_ctypes("/opt/axon/libaxon_pjrt.so")
    mod = types.ModuleType("antenv.axon_hooks")
    mod._hook = hook
    mod.get_axon_ntff_profile_hook = lambda: mod._hook
    mod.set_axon_ntff_profile_hook = lambda h: setattr(mod, "_hook", h)
    _sys.modules["antenv.axon_hooks"] = mod
    antenv.axon_hooks = mod


def kernel(X, alpha):
    global LAST_RESULT
    X = np.asarray(X, dtype=np.float32)
    a = float(np.asarray(alpha, dtype=np.float32).reshape(()))
    assert abs(a - 1.5) < 1e-6, f"unsupported alpha={a}"

    orig_shape = X.shape
    Xf = np.ascontiguousarray(X.reshape(-1, D))
    rows_total = Xf.shape[0]
    assert rows_total % N_CORES == 0
    rows = rows_total // N_CORES
    shards = np.split(Xf, N_CORES, axis=0)

    nc = _get_nc(rows)
    in_maps = [{"X": np.ascontiguousarray(s.astype(np.float16))}
               for s in shards]
    if TRACE:
        _ensure_ntff_hook()
    res = None
    for attempt in range(3):
        try:
            res = run_bass_kernel_spmd(nc, in_maps, list(range(N_CORES)),
                                       trace=TRACE)
            break
        except Exception:
            if attempt == 2:
                raise
            import time
            time.sleep(5.0)
    LAST_RESULT = res
    out = np.concatenate([np.asarray(r["OUT"]) for r in res.results], axis=0)
    return np.ascontiguousarray(out.astype(np.float32).reshape(orig_shape))
